# revision 13
# baseline (speedup 1.0000x reference)
"""AdaConv kernel for 8 TRN2 NeuronCores — data-parallel over batch.

Math (verified against the reference):
  The reference's per-sample grouped convs collapse:
    - depthwise conv output is identical across the 8 output channels of each
      group: D[n,g,h,w] = sum_{j,t} d[n,j,t] * xpad[n,8g+j,(h,w)+t]
    - pointwise 1x1 grouped conv collapses to a per-sample scalar
      S[n] = sum_j (s_d @ pk_w.T + pk_b)[n,j]
    - out = leaky(D[n,c//8]*S[n] + bias[n,c]) * (x - mean)/std  (instance norm)

  Stencil strategy (per core: 2 samples, 8 tiles of 128 channels):
    stage1: for each tap t<8, a column-tiled matmul (M=32 pair per 32-col
      strip, tile_position=(0,32k)) computes the PRE-SHIFTED partial sum
      z[(t,g), p] = sum_j d[j,t] x[8g+j, p+off_t] -> PSUM [128, win].
      The 4 strips run concurrently on disjoint PE sub-arrays, so the 8
      taps cost ~2 column-passes instead of 8.
    z is evicted to SBUF (bf16), then stage2 needs only TWO accumulating
      matmuls per output window: W2 (0/1 mask) @ z  +  W3 @ x (9th tap),
      both with uniform rhs offsets. Output is already replicated over the
      8 channels of each group; ScalarE evicts with fused leaky(S*D+bias).

  Layout: padded rows are 68 wide ([junk, reflectL, 64 data, reflectR,
  junk]) so every data row starts at an even bf16 offset -> DVE 4x/2x
  modes for the pad-copy (fused with sum accum), x^2 (fused with sumsq
  accum), xn, and final multiply. Input is loaded unpadded with an
  f32->bf16 cast in the DMA (SWDGE); reflection ring is built on-chip.
"""

import numpy as np
import ml_dtypes
from contextlib import ExitStack

import concourse.bass as bass
import concourse.tile as tile
from concourse import bacc, mybir
from concourse.bass_utils import run_bass_kernel_spmd

F32 = mybir.dt.float32
BF16 = mybir.dt.bfloat16
AF = mybir.ActivationFunctionType
ALU = mybir.AluOpType
AX = mybir.AxisListType

N_CORES = 8
NSAMP = 2            # samples per core
CH = 512
H = W = 64
PW = 68              # stored padded row width (2 junk cols for alignment)
PH = 66              # padded rows
PPXA = PW * PH + 4   # 4492, +4 so the last stencil read can over-read
ZLEN = 4352          # 64*68: pre-shifted z length (incl. 4-col seams)
# tap offsets into the stored layout: stored[h+kh, w+kw+1+...]
OFF = [kh * PW + kw + 1 for kh in range(3) for kw in range(3)]  # [1..139]
OFF8 = OFF[8]        # 139, the direct (2,2) tap
WIN2 = 7 * PW        # 476: stage2 window (7 output rows)

LAST_RESULTS = None  # BassKernelResults of the most recent run (for test.py)
_CACHE = {}


def _build():
    nc = bacc.Bacc("TRN2", target_bir_lowering=False, debug=False)

    xraw_d = nc.dram_tensor("xraw", [8, 128, H * W], F32, kind="ExternalInput")
    style_d = nc.dram_tensor("style", [128, NSAMP, 4, 16], F32, kind="ExternalInput")
    dwT_d = nc.dram_tensor("dwT", [128, 2, 2, 4, 8], F32, kind="ExternalInput")
    dwb_d = nc.dram_tensor("dwb", [8, 1], F32, kind="ExternalInput")
    pbT_d = nc.dram_tensor("pbT", [128, 4, 512], F32, kind="ExternalInput")
    pbb_d = nc.dram_tensor("pbb", [128, 4], F32, kind="ExternalInput")
    pkwT_d = nc.dram_tensor("pkwT", [128, 4, 8], F32, kind="ExternalInput")
    pkb_d = nc.dram_tensor("pkb", [1, 8], F32, kind="ExternalInput")
    mask_d = nc.dram_tensor("mask", [128, 128], BF16, kind="ExternalInput")
    maskh_d = nc.dram_tensor("maskh", [128, 2, 32], BF16, kind="ExternalInput")
    w2_d = nc.dram_tensor("w2", [128, 128], BF16, kind="ExternalInput")
    repl8_d = nc.dram_tensor("repl8", [8, 128], F32, kind="ExternalInput")
    out_d = nc.dram_tensor("out", [8, 128, H * W], BF16, kind="ExternalOutput")

    with tile.TileContext(nc) as tc, ExitStack() as ctx:
        const = ctx.enter_context(tc.tile_pool(name="const", bufs=1))
        small = ctx.enter_context(tc.tile_pool(name="small", bufs=1))
        rawp = ctx.enter_context(tc.tile_pool(name="raw", bufs=3))
        xpadp = ctx.enter_context(tc.tile_pool(name="xpad", bufs=6))
        junkp = ctx.enter_context(tc.tile_pool(name="junk", bufs=2))
        zp = ctx.enter_context(tc.tile_pool(name="z", bufs=2))
        predp = ctx.enter_context(tc.tile_pool(name="pred", bufs=5))
        xnp = ctx.enter_context(tc.tile_pool(name="xn", bufs=2))
        outp = ctx.enter_context(tc.tile_pool(name="outp", bufs=2))
        psZ = ctx.enter_context(tc.tile_pool(name="psZ", bufs=2, space="PSUM"))
        psD = ctx.enter_context(tc.tile_pool(name="psD", bufs=2, space="PSUM"))

        # ---- first content loads go out before the params ----
        raws = []
        for ts in range(2):
            raw = rawp.tile([128, H * W], BF16, tag="raw")
            raws.append(raw)
            nc.gpsimd.dma_start(raw[:], xraw_d[ts][:])  # f32 -> bf16 cast

        # ---- constants / params (single coalesced DMA each) ----
        style_sb = const.tile([128, NSAMP, 4, 16], F32)
        nc.sync.dma_start(style_sb[:], style_d[:])
        dwT_sb = const.tile([128, 2, 2, 4, 8], F32)
        nc.sync.dma_start(dwT_sb[:], dwT_d[:])
        dwb_sb = const.tile([8, 1], F32)
        nc.sync.dma_start(dwb_sb[:], dwb_d[:])
        repl8_sb = const.tile([8, 128], F32)
        nc.sync.dma_start(repl8_sb[:], repl8_d[:])
        maskh_sb = const.tile([128, 2, 32], BF16)
        nc.sync.dma_start(maskh_sb[:], maskh_d[:])
        mask_sb = const.tile([128, 128], BF16)
        nc.sync.dma_start(mask_sb[:], mask_d[:])
        w2_sb = const.tile([128, 128], BF16)
        nc.sync.dma_start(w2_sb[:], w2_d[:])
        pkb_sb = const.tile([1, 8], F32)
        nc.scalar.dma_start(pkb_sb[:], pkb_d[:])
        pbb_sb = const.tile([128, 4], F32)
        nc.scalar.dma_start(pbb_sb[:], pbb_d[:])
        pbT_sb = const.tile([128, 4, 512], F32)
        nc.scalar.dma_start(pbT_sb[:], pbT_d[:])
        pkwT_sb = const.tile([128, 4, 8], F32)
        nc.scalar.dma_start(pkwT_sb[:], pkwT_d[:])

        # ---- prologue: kernel-predictor math (all tiny, f32) ----
        W1_sb = const.tile([128, NSAMP, 8, 32], BF16)     # stage1 pair weights
        W3_sb = const.tile([128, NSAMP, 128], BF16)       # direct 9th-tap weight
        bias_sb = const.tile([128, 4, NSAMP], F32)        # per-channel bias [mt, s]
        Sb_sb = const.tile([128, NSAMP], F32)             # S[n] bcast to 128 parts
        d_sb = small.tile([8, NSAMP, 9], F32)
        dcol_sb = small.tile([128, NSAMP, 9], F32)
        ssum_sb = small.tile([128, 4, NSAMP], F32)        # style sums [kt, s]
        pkwsum_sb = small.tile([128, 4], F32)
        pkbsum_sb = small.tile([1, 1], F32)
        S_sb = small.tile([1, NSAMP], F32)

        eps_sb = const.tile([128, 1], F32)
        nc.vector.memset(eps_sb[:], 1e-5)

        nc.vector.tensor_reduce(pkbsum_sb[:], pkb_sb[:], axis=AX.X, op=ALU.add)
        for kt in range(4):
            nc.vector.tensor_reduce(
                pkwsum_sb[:, kt:kt + 1], pkwT_sb[:, kt, :], axis=AX.X, op=ALU.add)

        for s in range(NSAMP):
            # d = leaky(conv2x2(style, dw_w) + dw_b):  16 accumulating matmuls
            ps_d = psZ.tile([8, 9], F32, tag="psz")
            i = 0
            for ky in range(2):
                for kx in range(2):
                    for kt in range(4):
                        rhs = style_sb[:, s, kt, :].rearrange(
                            "p (y x) -> p y x", x=4)[:, ky:ky + 3, kx:kx + 3]
                        nc.tensor.matmul(
                            ps_d[:], dwT_sb[:, ky, kx, kt, :], rhs,
                            start=(i == 0), stop=(i == 15))
                        i += 1
            nc.scalar.activation(
                d_sb[:, s, :], ps_d[:], AF.Lrelu, bias=dwb_sb[:], alpha=0.01)

            # replicate d over the 128-channel pattern: dcol[c,t] = d[c%8,t]
            ps_dc = psZ.tile([128, 9], F32, tag="psz")
            nc.tensor.matmul(ps_dc[:], repl8_sb[:], d_sb[:, s, :])
            nc.vector.tensor_copy(dcol_sb[:, s, :], ps_dc[:])

            # stage1 pair weights: W1[:, s, t, :] = maskh[t%2] * d-col t
            for t in range(8):
                nc.vector.tensor_scalar(
                    W1_sb[:, s, t, :], maskh_sb[:, t % 2, :],
                    dcol_sb[:, s, t:t + 1], None, ALU.mult)
            # direct tap weight: W3 = mask * dcol[:, 8]
            nc.vector.tensor_scalar(
                W3_sb[:, s, :], mask_sb[:], dcol_sb[:, s, 8:9], None, ALU.mult)

            # style spatial sums (s_d * 16)
            for kt in range(4):
                nc.vector.tensor_reduce(
                    ssum_sb[:, kt, s:s + 1], style_sb[:, s, kt, :],
                    axis=AX.X, op=ALU.add)

        # bias[c] = s_d @ pb_w[c] + pb_b[c]   (both samples batched, N=2)
        for mt in range(4):
            ps_b = psZ.tile([128, NSAMP], F32, tag="psz")
            for kt in range(4):
                nc.tensor.matmul(
                    ps_b[:], pbT_sb[:, kt, mt * 128:(mt + 1) * 128],
                    ssum_sb[:, kt, :], start=(kt == 0), stop=(kt == 3))
            nc.scalar.activation(
                bias_sb[:, mt, :], ps_b[:], AF.Identity,
                bias=pbb_sb[:, mt:mt + 1], scale=1.0 / 16.0)

        # S = s_d @ pkw_sum + sum(pk_b)   (both samples, N=2)
        ps_S = psZ.tile([1, NSAMP], F32, tag="psz")
        for kt in range(4):
            nc.tensor.matmul(
                ps_S[:], pkwsum_sb[:, kt:kt + 1], ssum_sb[:, kt, :],
                start=(kt == 0), stop=(kt == 3))
        nc.scalar.activation(
            S_sb[:], ps_S[:], AF.Identity, bias=pkbsum_sb[:], scale=1.0 / 16.0)
        nc.gpsimd.partition_broadcast(Sb_sb[:], S_sb[:])

        # ---- instance-norm statistics (batched finalize every 4 tiles) ----
        s_all = small.tile([128, 8], F32)     # per-ts sum(x)
        q_all = small.tile([128, 8], F32)     # per-ts sum(x^2)
        rstd_all = small.tile([128, 8], F32)
        nmr_all = small.tile([128, 8], F32)
        t0_all = small.tile([128, 8], F32)
        u_all = small.tile([128, 8], F32)
        stdv_all = small.tile([128, 8], F32)

        def finalize_stats(c0):
            sl = slice(c0, c0 + 4)
            nc.vector.tensor_tensor(t0_all[:, sl], s_all[:, sl], s_all[:, sl],
                                    ALU.mult)
            nc.vector.tensor_scalar(
                u_all[:, sl], t0_all[:, sl], -1.0 / 4096.0, None, ALU.mult)
            nc.vector.tensor_tensor(u_all[:, sl], u_all[:, sl], q_all[:, sl],
                                    ALU.add)
            nc.scalar.activation(
                stdv_all[:, sl], u_all[:, sl], AF.Sqrt, scale=1.0 / 4095.0,
                bias=eps_sb[:])
            nc.vector.reciprocal(rstd_all[:, sl], stdv_all[:, sl])
            nc.vector.tensor_tensor(nmr_all[:, sl], rstd_all[:, sl],
                                    s_all[:, sl], ALU.mult)
            nc.vector.tensor_scalar(
                nmr_all[:, sl], nmr_all[:, sl], -1.0 / 4096.0, None, ALU.mult)

        # ---- main loop over the 8 sample-channel tiles ----
        preds = []
        xpads = []
        zs = []

        def emit_norm(k):
            xpvk = xpads[k][:, :PW * PH].rearrange("p (h w) -> p h w", w=PW)
            xn = xnp.tile([128, H * W], BF16, tag="xn")
            nc.vector.tensor_scalar(
                xn[:].rearrange("p (h w) -> p h w", w=W), xpvk[:, 1:65, 2:66],
                rstd_all[:, k:k + 1], nmr_all[:, k:k + 1],
                ALU.mult, ALU.add)
            out_sb = outp.tile([128, H * W], BF16, tag="out")
            nc.vector.tensor_tensor(out_sb[:], preds[k][:], xn[:], ALU.mult)
            for c in range(2):
                lo, hi = c * 2048, (c + 1) * 2048
                nc.sync.dma_start(out_d[k][:, lo:hi], out_sb[:, lo:hi])

        def emit_load(k):
            nraw = rawp.tile([128, H * W], BF16, tag="raw")
            nc.gpsimd.dma_start(nraw[:], xraw_d[k][:])  # f32 -> bf16 cast
            raws.append(nraw)

        def emit_pad(ts):
            raw = raws[ts]
            rawv = raw[:].rearrange("p (h w) -> p h w", w=W)
            xpad = xpadp.tile([128, PPXA], BF16, tag="xpad")
            xpv = xpad[:, :PW * PH].rearrange("p (h w) -> p h w", w=PW)
            nc.vector.tensor_copy(xpv[:, 1:65, 2:66], rawv)
            nc.vector.tensor_copy(xpv[:, 1:65, 1:2], rawv[:, :, 1:2])
            nc.vector.tensor_copy(xpv[:, 1:65, 66:67], rawv[:, :, 62:63])
            nc.vector.tensor_copy(xpv[:, 0:1, 1:67], xpv[:, 2:3, 1:67])
            nc.vector.tensor_copy(xpv[:, 65:66, 1:67], xpv[:, 63:64, 1:67])
            xpads.append(xpad)

        def emit_stage1(ts):
            s = ts // 4
            xpad = xpads[ts]
            z = zp.tile([128, ZLEN], BF16, tag="z")
            zs.append(z)
            for v in range(5):
                ncols = 1024 if v < 4 else 256
                psz = psZ.tile([128, 1024], F32, tag="psz")
                for half in range(2):
                    w = 2 * v + half
                    if w > 8:
                        break
                    nv = 512 if w < 8 else 256
                    vb = 512 * w
                    # wave order: all 4 strips' first-of-pair, then seconds
                    for b in range(2):
                        for k in range(4):
                            t = 2 * k + b
                            o = OFF[t]
                            nc.tensor.matmul(
                                psz[32 * k:32 * k + 32,
                                    512 * half:512 * half + nv],
                                W1_sb[:, s, t, :],
                                xpad[:, vb + o:vb + o + nv],
                                start=(b == 0), stop=(b == 1),
                                tile_position=(0, 32 * k))
                if v in (1, 3):
                    nc.vector.tensor_copy(z[:, 1024 * v:1024 * v + ncols],
                                          psz[:, :ncols])
                else:
                    nc.scalar.copy(z[:, 1024 * v:1024 * v + ncols],
                                   psz[:, :ncols])

        def emit_stage2(ts):
            s = ts // 4
            q = ts % 4
            xpad = xpads[ts]
            z = zs[ts]
            pred = predp.tile([128, H * W], BF16, tag="pred")
            preds.append(pred)
            for f in range(5):
                psd = psD.tile([128, 1024], F32, tag="psd")
                slots = [sl for sl in range(2) if 2 * f + sl <= 9]
                # batch by weight: both slots' W2 matmuls, then both W3
                for b in range(2):
                    for slot in slots:
                        w2 = 2 * f + slot
                        n = WIN2 if w2 < 9 else PW
                        wb = WIN2 * w2
                        dst = psd[:, 512 * slot:512 * slot + n]
                        if b == 0:
                            nc.tensor.matmul(dst, w2_sb[:], z[:, wb:wb + n],
                                             start=True, stop=False)
                        else:
                            nc.tensor.matmul(
                                dst, W3_sb[:, s, :],
                                xpad[:, wb + OFF8:wb + OFF8 + n],
                                start=False, stop=True)
                # fused leaky(S*D + bias), strided to skip the 4-col seams
                if f < 4:
                    srcv = psd[:].rearrange("p (k x) -> p k x", x=512) \
                        [:, :, :WIN2].rearrange(
                            "p k (r w) -> p k r w", w=PW)[:, :, :, :64]
                    dsts = [(srcv, pred[:, 896 * f:896 * (f + 1)])]
                else:
                    src0 = psd[:, :WIN2].rearrange(
                        "p (r w) -> p r w", w=PW)[:, :, :64]
                    dsts = [(src0, pred[:, 3584:4032]),
                            (psd[:, 512:512 + 64], pred[:, 4032:4096])]
                for srcv, dstv in dsts:
                    nc.scalar.activation(
                        dstv, srcv, AF.Lrelu,
                        bias=bias_sb[:, q, s:s + 1], scale=Sb_sb[:, s:s + 1],
                        alpha=0.01)

        def emit_stats(ts):
            raw = raws[ts]
            # sum(x^2): square, 3 in-place accumulate-DMA folds, small reduce
            junk = junkp.tile([128, H * W], BF16, tag="junk")
            nc.vector.tensor_tensor(junk[:], raw[:], raw[:], ALU.mult)
            nc.gpsimd.dma_start(junk[:, :2048], junk[:, 2048:4096],
                                accum_op=ALU.add)
            nc.gpsimd.dma_start(junk[:, :1024], junk[:, 1024:2048],
                                accum_op=ALU.add)
            nc.gpsimd.dma_start(junk[:, :512], junk[:, 512:1024],
                                accum_op=ALU.add)
            nc.vector.tensor_reduce(
                q_all[:, ts:ts + 1], junk[:, :512], axis=AX.X, op=ALU.add)
            # sum(x): same trick, folding raw in place (raw is dead after)
            nc.gpsimd.dma_start(raw[:, :2048], raw[:, 2048:4096],
                                accum_op=ALU.add)
            nc.gpsimd.dma_start(raw[:, :1024], raw[:, 1024:2048],
                                accum_op=ALU.add)
            nc.gpsimd.dma_start(raw[:, :512], raw[:, 512:1024],
                                accum_op=ALU.add)
            nc.vector.tensor_reduce(
                s_all[:, ts:ts + 1], raw[:, :512], axis=AX.X, op=ALU.add)

        # stage2 lags stage1 by one tile so its z operand is always fully
        # evicted; normalization lags four so the stats finalize is upstream.
        for it in range(10):
            if it < 8:
                if 1 <= it and it + 1 < 8:
                    emit_load(it + 1)
                emit_pad(it)
                emit_stage1(it)
            if 1 <= it <= 8:
                emit_stage2(it - 1)
            if 1 <= it <= 8:
                emit_stats(it - 1)
            if it == 4:
                finalize_stats(0)
            if 5 <= it <= 8:
                emit_norm(it - 5)
            if it == 9:
                finalize_stats(4)
                for k in range(4, 8):
                    emit_norm(k)

    nc.compile()
    return nc


def _host_prep(style_encoding, content_in, dw_w, dw_b, pk_w, pk_b, pb_w, pb_b):
    """Shard + lay out inputs for the 8 cores (layout only, no math)."""
    f32 = np.float32
    bf16 = ml_dtypes.bfloat16
    common = {
        # dwT[c_lo, ky, kx, kt, p] = dw_w[p, kt*128+c_lo, ky, kx]
        "dwT": np.ascontiguousarray(
            dw_w.reshape(8, 4, 128, 2, 2).transpose(2, 3, 4, 1, 0), f32),
        "dwb": np.ascontiguousarray(dw_b.reshape(8, 1), f32),
        # pbT[c_lo, kt, m] = pb_w[m, kt*128+c_lo]
        "pbT": np.ascontiguousarray(
            pb_w.T.reshape(4, 128, 512).transpose(1, 0, 2), f32),
        "pbb": np.ascontiguousarray(pb_b.reshape(4, 128).T, f32),
        "pkwT": np.ascontiguousarray(
            pk_w.T.reshape(4, 128, 8).transpose(1, 0, 2), f32),
        "pkb": np.ascontiguousarray(pk_b.reshape(1, 8), f32),
    }
    ii = np.arange(128)
    mask = ((ii[:, None] // 8) == (ii[None, :] // 8)).astype(bf16)
    common["mask"] = mask
    # maskh[c, half, g] = 1 if c//8 == g(+16*half pattern): the two 32-wide
    # halves used to build the stage1 pair weights [W1_t | 0] and [0 | W1_t]
    maskh = np.zeros((128, 2, 32), bf16)
    m16 = ((ii[:, None] // 8) == np.arange(16)[None, :]).astype(bf16)
    maskh[:, 0, 0:16] = m16
    maskh[:, 1, 16:32] = m16
    common["maskh"] = maskh
    # W2[zrow = 32k+16b+g, m] = 1 if g == m//8  (sums the 8 tap blocks)
    zg = (ii % 32) % 16
    common["w2"] = (zg[:, None] == (ii[None, :] // 8)).astype(bf16)
    common["repl8"] = (np.arange(8)[:, None] == (ii[None, :] % 8)).astype(f32)

    in_maps = []
    for i in range(N_CORES):
        x = content_in[NSAMP * i: NSAMP * (i + 1)]
        se = style_encoding[NSAMP * i: NSAMP * (i + 1)]
        in_maps.append({
            "xraw": np.ascontiguousarray(
                x.reshape(NSAMP * 4, 128, H * W), f32),
            # style[c_lo, s, kt, yx] = se[s, kt*128+c_lo, yx]
            "style": np.ascontiguousarray(
                se.reshape(NSAMP, 4, 128, 16).transpose(2, 0, 1, 3), f32),
            **common,
        })
    return in_maps


def kernel(style_encoding, content_in, dw_w, dw_b, pk_w, pk_b, pb_w, pb_b):
    global LAST_RESULTS
    import os
    if "nc" not in _CACHE:
        _CACHE["nc"] = _build()
    nc = _CACHE["nc"]
    in_maps = _host_prep(style_encoding, content_in, dw_w, dw_b,
                         pk_w, pk_b, pb_w, pb_b)
    res = run_bass_kernel_spmd(
        nc, in_maps, core_ids=list(range(N_CORES)),
        trace=bool(os.environ.get("ADACONV_TRACE")))
    LAST_RESULTS = res
    outs = []
    for i in range(N_CORES):
        o = np.asarray(res.results[i]["out"]).astype(np.float32)
        outs.append(o.reshape(NSAMP, 4, 128, 64, 64).reshape(NSAMP, CH, 64, 64))
    return np.concatenate(outs, axis=0)


# revision 14
# speedup vs baseline: 1.1342x; 1.1342x over previous
"""AdaConv kernel for 8 TRN2 NeuronCores — data-parallel over batch.

Math (verified against the reference):
  The reference's per-sample grouped convs collapse:
    - depthwise conv output is identical across the 8 output channels of each
      group: D[n,g,h,w] = sum_{j,t} d[n,j,t] * xpad[n,8g+j,(h,w)+t]
    - pointwise 1x1 grouped conv collapses to a per-sample scalar
      S[n] = sum_j (s_d @ pk_w.T + pk_b)[n,j]
    - out = leaky(D[n,c//8]*S[n] + bias[n,c]) * (x - mean)/std  (instance norm)

  Stencil strategy (per core: 2 samples, 8 tiles of 128 channels):
    stage1: for each tap t<8, a column-tiled matmul (M=32 pair per 32-col
      strip, tile_position=(0,32k)) computes the PRE-SHIFTED partial sum
      z[(t,g), p] = sum_j d[j,t] x[8g+j, p+off_t] -> PSUM [128, win].
      The 4 strips run concurrently on disjoint PE sub-arrays, so the 8
      taps cost ~2 column-passes instead of 8.
    z is evicted to SBUF (bf16), then stage2 needs only TWO accumulating
      matmuls per output window: W2 (0/1 mask) @ z  +  W3 @ x (9th tap),
      both with uniform rhs offsets. Output is already replicated over the
      8 channels of each group; ScalarE evicts with fused leaky(S*D+bias).

  Layout: padded rows are 68 wide ([junk, reflectL, 64 data, reflectR,
  junk]) so every data row starts at an even bf16 offset -> DVE 4x/2x
  modes for the pad-copy (fused with sum accum), x^2 (fused with sumsq
  accum), xn, and final multiply. Input is loaded unpadded with an
  f32->bf16 cast in the DMA (SWDGE); reflection ring is built on-chip.
"""

import numpy as np
import ml_dtypes
from contextlib import ExitStack

import concourse.bass as bass
import concourse.tile as tile
from concourse import bacc, mybir
from concourse.bass_utils import run_bass_kernel_spmd

F32 = mybir.dt.float32
BF16 = mybir.dt.bfloat16
AF = mybir.ActivationFunctionType
ALU = mybir.AluOpType
AX = mybir.AxisListType

N_CORES = 8
NSAMP = 2            # samples per core
CH = 512
H = W = 64
PW = 68              # stored padded row width (2 junk cols for alignment)
PH = 66              # padded rows
PPXA = PW * PH + 4   # 4492, +4 so the last stencil read can over-read
ZLEN = 4352          # 64*68: pre-shifted z length (incl. 4-col seams)
# tap offsets into the stored layout: stored[h+kh, w+kw+1+...]
OFF = [kh * PW + kw + 1 for kh in range(3) for kw in range(3)]  # [1..139]
OFF8 = OFF[8]        # 139, the direct (2,2) tap
WIN2 = 7 * PW        # 476: stage2 window (7 output rows)

LAST_RESULTS = None  # BassKernelResults of the most recent run (for test.py)
_CACHE = {}


def _build():
    nc = bacc.Bacc("TRN2", target_bir_lowering=False, debug=False)

    xraw_d = nc.dram_tensor("xraw", [8, 128, H * W], F32, kind="ExternalInput")
    style_d = nc.dram_tensor("style", [128, NSAMP, 4, 16], F32, kind="ExternalInput")
    dwT_d = nc.dram_tensor("dwT", [128, 2, 2, 4, 8], F32, kind="ExternalInput")
    dwb_d = nc.dram_tensor("dwb", [8, 1], F32, kind="ExternalInput")
    pbT_d = nc.dram_tensor("pbT", [128, 4, 512], F32, kind="ExternalInput")
    pbb_d = nc.dram_tensor("pbb", [128, 4], F32, kind="ExternalInput")
    pkwT_d = nc.dram_tensor("pkwT", [128, 4, 8], F32, kind="ExternalInput")
    pkb_d = nc.dram_tensor("pkb", [1, 8], F32, kind="ExternalInput")
    mask_d = nc.dram_tensor("mask", [128, 128], BF16, kind="ExternalInput")
    maskh_d = nc.dram_tensor("maskh", [128, 2, 32], BF16, kind="ExternalInput")
    w2_d = nc.dram_tensor("w2", [128, 128], BF16, kind="ExternalInput")
    repl8_d = nc.dram_tensor("repl8", [8, 128], F32, kind="ExternalInput")
    out_d = nc.dram_tensor("out", [8, 128, H * W], BF16, kind="ExternalOutput")

    with tile.TileContext(nc) as tc, ExitStack() as ctx:
        const = ctx.enter_context(tc.tile_pool(name="const", bufs=1))
        small = ctx.enter_context(tc.tile_pool(name="small", bufs=1))
        rawp = ctx.enter_context(tc.tile_pool(name="raw", bufs=3))
        xpadp = ctx.enter_context(tc.tile_pool(name="xpad", bufs=6))
        junkp = ctx.enter_context(tc.tile_pool(name="junk", bufs=2))
        zp = ctx.enter_context(tc.tile_pool(name="z", bufs=2))
        predp = ctx.enter_context(tc.tile_pool(name="pred", bufs=5))
        xnp = ctx.enter_context(tc.tile_pool(name="xn", bufs=2))
        outp = ctx.enter_context(tc.tile_pool(name="outp", bufs=2))
        psZ = ctx.enter_context(tc.tile_pool(name="psZ", bufs=2, space="PSUM"))
        psD = ctx.enter_context(tc.tile_pool(name="psD", bufs=2, space="PSUM"))

        # ---- first content loads go out before the params ----
        raws = []
        for ts in range(2):
            raw = rawp.tile([128, H * W], BF16, tag="raw")
            raws.append(raw)
            nc.gpsimd.dma_start(raw[:], xraw_d[ts][:])  # f32 -> bf16 cast

        # ---- constants / params (single coalesced DMA each) ----
        style_sb = const.tile([128, NSAMP, 4, 16], F32)
        nc.sync.dma_start(style_sb[:], style_d[:])
        dwT_sb = const.tile([128, 2, 2, 4, 8], F32)
        nc.sync.dma_start(dwT_sb[:], dwT_d[:])
        dwb_sb = const.tile([8, 1], F32)
        nc.sync.dma_start(dwb_sb[:], dwb_d[:])
        repl8_sb = const.tile([8, 128], F32)
        nc.sync.dma_start(repl8_sb[:], repl8_d[:])
        maskh_sb = const.tile([128, 2, 32], BF16)
        nc.sync.dma_start(maskh_sb[:], maskh_d[:])
        mask_sb = const.tile([128, 128], BF16)
        nc.sync.dma_start(mask_sb[:], mask_d[:])
        w2_sb = const.tile([128, 128], BF16)
        nc.sync.dma_start(w2_sb[:], w2_d[:])
        pkb_sb = const.tile([1, 8], F32)
        nc.scalar.dma_start(pkb_sb[:], pkb_d[:])
        pbb_sb = const.tile([128, 4], F32)
        nc.scalar.dma_start(pbb_sb[:], pbb_d[:])
        pbT_sb = const.tile([128, 4, 512], F32)
        nc.scalar.dma_start(pbT_sb[:], pbT_d[:])
        pkwT_sb = const.tile([128, 4, 8], F32)
        nc.scalar.dma_start(pkwT_sb[:], pkwT_d[:])

        # ---- prologue: kernel-predictor math (all tiny, f32) ----
        W1_sb = const.tile([128, NSAMP, 8, 32], BF16)     # stage1 pair weights
        W3_sb = const.tile([128, NSAMP, 128], BF16)       # direct 9th-tap weight
        bias_sb = const.tile([128, 4, NSAMP], F32)        # per-channel bias [mt, s]
        Sb_sb = const.tile([128, NSAMP], F32)             # S[n] bcast to 128 parts
        d_sb = small.tile([8, NSAMP, 9], F32)
        dcol_sb = small.tile([128, NSAMP, 9], F32)
        ssum_sb = small.tile([128, 4, NSAMP], F32)        # style sums [kt, s]
        pkwsum_sb = small.tile([128, 4], F32)
        pkbsum_sb = small.tile([1, 1], F32)
        S_sb = small.tile([1, NSAMP], F32)

        eps_sb = const.tile([128, 1], F32)
        nc.vector.memset(eps_sb[:], 1e-5)

        nc.vector.tensor_reduce(pkbsum_sb[:], pkb_sb[:], axis=AX.X, op=ALU.add)
        for kt in range(4):
            nc.vector.tensor_reduce(
                pkwsum_sb[:, kt:kt + 1], pkwT_sb[:, kt, :], axis=AX.X, op=ALU.add)

        for s in range(NSAMP):
            # d = leaky(conv2x2(style, dw_w) + dw_b):  16 accumulating matmuls
            ps_d = psZ.tile([8, 9], F32, tag="psz")
            i = 0
            for ky in range(2):
                for kx in range(2):
                    for kt in range(4):
                        rhs = style_sb[:, s, kt, :].rearrange(
                            "p (y x) -> p y x", x=4)[:, ky:ky + 3, kx:kx + 3]
                        nc.tensor.matmul(
                            ps_d[:], dwT_sb[:, ky, kx, kt, :], rhs,
                            start=(i == 0), stop=(i == 15))
                        i += 1
            nc.scalar.activation(
                d_sb[:, s, :], ps_d[:], AF.Lrelu, bias=dwb_sb[:], alpha=0.01)

            # replicate d over the 128-channel pattern: dcol[c,t] = d[c%8,t]
            ps_dc = psZ.tile([128, 9], F32, tag="psz")
            nc.tensor.matmul(ps_dc[:], repl8_sb[:], d_sb[:, s, :])
            nc.vector.tensor_copy(dcol_sb[:, s, :], ps_dc[:])

            # stage1 pair weights: W1[:, s, t, :] = maskh[t%2] * d-col t
            for t in range(8):
                nc.vector.tensor_scalar(
                    W1_sb[:, s, t, :], maskh_sb[:, t % 2, :],
                    dcol_sb[:, s, t:t + 1], None, ALU.mult)
            # direct tap weight: W3 = mask * dcol[:, 8]
            nc.vector.tensor_scalar(
                W3_sb[:, s, :], mask_sb[:], dcol_sb[:, s, 8:9], None, ALU.mult)

            # style spatial sums (s_d * 16)
            for kt in range(4):
                nc.vector.tensor_reduce(
                    ssum_sb[:, kt, s:s + 1], style_sb[:, s, kt, :],
                    axis=AX.X, op=ALU.add)

        # bias[c] = s_d @ pb_w[c] + pb_b[c]   (both samples batched, N=2)
        for mt in range(4):
            ps_b = psZ.tile([128, NSAMP], F32, tag="psz")
            for kt in range(4):
                nc.tensor.matmul(
                    ps_b[:], pbT_sb[:, kt, mt * 128:(mt + 1) * 128],
                    ssum_sb[:, kt, :], start=(kt == 0), stop=(kt == 3))
            nc.scalar.activation(
                bias_sb[:, mt, :], ps_b[:], AF.Identity,
                bias=pbb_sb[:, mt:mt + 1], scale=1.0 / 16.0)

        # S = s_d @ pkw_sum + sum(pk_b)   (both samples, N=2)
        ps_S = psZ.tile([1, NSAMP], F32, tag="psz")
        for kt in range(4):
            nc.tensor.matmul(
                ps_S[:], pkwsum_sb[:, kt:kt + 1], ssum_sb[:, kt, :],
                start=(kt == 0), stop=(kt == 3))
        nc.scalar.activation(
            S_sb[:], ps_S[:], AF.Identity, bias=pkbsum_sb[:], scale=1.0 / 16.0)
        nc.gpsimd.partition_broadcast(Sb_sb[:], S_sb[:])

        # ---- instance-norm statistics (batched finalize every 4 tiles) ----
        s_all = small.tile([128, 8], F32)     # per-ts sum(x)
        q_all = small.tile([128, 8], F32)     # per-ts sum(x^2)
        rstd_all = small.tile([128, 8], F32)
        nmr_all = small.tile([128, 8], F32)
        t0_all = small.tile([128, 8], F32)
        u_all = small.tile([128, 8], F32)
        stdv_all = small.tile([128, 8], F32)

        def finalize_stats(c0):
            sl = slice(c0, c0 + 4)
            nc.vector.tensor_tensor(t0_all[:, sl], s_all[:, sl], s_all[:, sl],
                                    ALU.mult)
            nc.vector.tensor_scalar(
                u_all[:, sl], t0_all[:, sl], -1.0 / 4096.0, None, ALU.mult)
            nc.vector.tensor_tensor(u_all[:, sl], u_all[:, sl], q_all[:, sl],
                                    ALU.add)
            nc.scalar.activation(
                stdv_all[:, sl], u_all[:, sl], AF.Sqrt, scale=1.0 / 4095.0,
                bias=eps_sb[:])
            nc.vector.reciprocal(rstd_all[:, sl], stdv_all[:, sl])
            nc.vector.tensor_tensor(nmr_all[:, sl], rstd_all[:, sl],
                                    s_all[:, sl], ALU.mult)
            nc.vector.tensor_scalar(
                nmr_all[:, sl], nmr_all[:, sl], -1.0 / 4096.0, None, ALU.mult)

        # ---- main loop over the 8 sample-channel tiles ----
        preds = []
        xpads = []
        zs = []

        def emit_norm(k):
            xpvk = xpads[k][:, :PW * PH].rearrange("p (h w) -> p h w", w=PW)
            xn = xnp.tile([128, H * W], BF16, tag="xn")
            nc.vector.tensor_scalar(
                xn[:].rearrange("p (h w) -> p h w", w=W), xpvk[:, 1:65, 2:66],
                rstd_all[:, k:k + 1], nmr_all[:, k:k + 1],
                ALU.mult, ALU.add)
            out_sb = outp.tile([128, H * W], BF16, tag="out")
            nc.vector.tensor_tensor(out_sb[:], preds[k][:], xn[:], ALU.mult)
            for c in range(2):
                lo, hi = c * 2048, (c + 1) * 2048
                nc.sync.dma_start(out_d[k][:, lo:hi], out_sb[:, lo:hi])

        def emit_load(k):
            nraw = rawp.tile([128, H * W], BF16, tag="raw")
            nc.gpsimd.dma_start(nraw[:], xraw_d[k][:])  # f32 -> bf16 cast
            raws.append(nraw)

        def emit_pad(ts):
            raw = raws[ts]
            rawv = raw[:].rearrange("p (h w) -> p h w", w=W)
            xpad = xpadp.tile([128, PPXA], BF16, tag="xpad")
            xpv = xpad[:, :PW * PH].rearrange("p (h w) -> p h w", w=PW)
            nc.vector.tensor_copy(xpv[:, 1:65, 2:66], rawv)
            nc.vector.tensor_copy(xpv[:, 1:65, 1:2], rawv[:, :, 1:2])
            nc.vector.tensor_copy(xpv[:, 1:65, 66:67], rawv[:, :, 62:63])
            nc.vector.tensor_copy(xpv[:, 0:1, 1:67], xpv[:, 2:3, 1:67])
            nc.vector.tensor_copy(xpv[:, 65:66, 1:67], xpv[:, 63:64, 1:67])
            xpads.append(xpad)

        def emit_stage1(ts):
            s = ts // 4
            xpad = xpads[ts]
            z = zp.tile([128, ZLEN], BF16, tag="z")
            zs.append(z)
            for v in range(5):
                ncols = 1024 if v < 4 else 256
                psz = psZ.tile([128, 1024], F32, tag="psz")
                for half in range(2):
                    w = 2 * v + half
                    if w > 8:
                        break
                    nv = 512 if w < 8 else 256
                    vb = 512 * w
                    # wave order: all 4 strips' first-of-pair, then seconds
                    for b in range(2):
                        for k in range(4):
                            t = 2 * k + b
                            o = OFF[t]
                            nc.tensor.matmul(
                                psz[32 * k:32 * k + 32,
                                    512 * half:512 * half + nv],
                                W1_sb[:, s, t, :],
                                xpad[:, vb + o:vb + o + nv],
                                start=(b == 0), stop=(b == 1),
                                tile_position=(0, 32 * k))
                nc.scalar.copy(z[:, 1024 * v:1024 * v + ncols],
                               psz[:, :ncols])

        def emit_stage2(ts):
            s = ts // 4
            q = ts % 4
            xpad = xpads[ts]
            z = zs[ts]
            pred = predp.tile([128, H * W], BF16, tag="pred")
            preds.append(pred)
            for f in range(5):
                psd = psD.tile([128, 1024], F32, tag="psd")
                slots = [sl for sl in range(2) if 2 * f + sl <= 9]
                # batch by weight: both slots' W2 matmuls, then both W3
                for b in range(2):
                    for slot in slots:
                        w2 = 2 * f + slot
                        n = WIN2 if w2 < 9 else PW
                        wb = WIN2 * w2
                        dst = psd[:, 512 * slot:512 * slot + n]
                        if b == 0:
                            nc.tensor.matmul(dst, w2_sb[:], z[:, wb:wb + n],
                                             start=True, stop=False)
                        else:
                            nc.tensor.matmul(
                                dst, W3_sb[:, s, :],
                                xpad[:, wb + OFF8:wb + OFF8 + n],
                                start=False, stop=True)
                # fused leaky(S*D + bias), strided to skip the 4-col seams
                if f < 4:
                    srcv = psd[:].rearrange("p (k x) -> p k x", x=512) \
                        [:, :, :WIN2].rearrange(
                            "p k (r w) -> p k r w", w=PW)[:, :, :, :64]
                    dsts = [(srcv, pred[:, 896 * f:896 * (f + 1)])]
                else:
                    src0 = psd[:, :WIN2].rearrange(
                        "p (r w) -> p r w", w=PW)[:, :, :64]
                    dsts = [(src0, pred[:, 3584:4032]),
                            (psd[:, 512:512 + 64], pred[:, 4032:4096])]
                for srcv, dstv in dsts:
                    nc.scalar.activation(
                        dstv, srcv, AF.Lrelu,
                        bias=bias_sb[:, q, s:s + 1], scale=Sb_sb[:, s:s + 1],
                        alpha=0.01)

        def emit_stats(ts):
            raw = raws[ts]
            # sum(x^2): square, 3 in-place accumulate-DMA folds, small reduce
            junk = junkp.tile([128, H * W], BF16, tag="junk")
            nc.vector.tensor_tensor(junk[:], raw[:], raw[:], ALU.mult)
            nc.gpsimd.dma_start(junk[:, :2048], junk[:, 2048:4096],
                                accum_op=ALU.add)
            nc.gpsimd.dma_start(junk[:, :1024], junk[:, 1024:2048],
                                accum_op=ALU.add)
            nc.gpsimd.dma_start(junk[:, :512], junk[:, 512:1024],
                                accum_op=ALU.add)
            nc.vector.tensor_reduce(
                q_all[:, ts:ts + 1], junk[:, :512], axis=AX.X, op=ALU.add)
            # sum(x): same trick, folding raw in place (raw is dead after)
            nc.gpsimd.dma_start(raw[:, :2048], raw[:, 2048:4096],
                                accum_op=ALU.add)
            nc.gpsimd.dma_start(raw[:, :1024], raw[:, 1024:2048],
                                accum_op=ALU.add)
            nc.gpsimd.dma_start(raw[:, :512], raw[:, 512:1024],
                                accum_op=ALU.add)
            nc.vector.tensor_reduce(
                s_all[:, ts:ts + 1], raw[:, :512], axis=AX.X, op=ALU.add)

        # stage2 lags stage1 by one tile so its z operand is always fully
        # evicted; normalization lags four so the stats finalize is upstream.
        for it in range(10):
            if it < 8:
                if 1 <= it and it + 1 < 8:
                    emit_load(it + 1)
                emit_pad(it)
            if 1 <= it <= 8:
                emit_stage2(it - 1)
            if it < 8:
                emit_stage1(it)
            if 1 <= it <= 8:
                emit_stats(it - 1)
            if it == 4:
                finalize_stats(0)
            if 5 <= it <= 8:
                emit_norm(it - 5)
            if it == 9:
                finalize_stats(4)
                for k in range(4, 8):
                    emit_norm(k)

    nc.compile()
    return nc


def _host_prep(style_encoding, content_in, dw_w, dw_b, pk_w, pk_b, pb_w, pb_b):
    """Shard + lay out inputs for the 8 cores (layout only, no math)."""
    f32 = np.float32
    bf16 = ml_dtypes.bfloat16
    common = {
        # dwT[c_lo, ky, kx, kt, p] = dw_w[p, kt*128+c_lo, ky, kx]
        "dwT": np.ascontiguousarray(
            dw_w.reshape(8, 4, 128, 2, 2).transpose(2, 3, 4, 1, 0), f32),
        "dwb": np.ascontiguousarray(dw_b.reshape(8, 1), f32),
        # pbT[c_lo, kt, m] = pb_w[m, kt*128+c_lo]
        "pbT": np.ascontiguousarray(
            pb_w.T.reshape(4, 128, 512).transpose(1, 0, 2), f32),
        "pbb": np.ascontiguousarray(pb_b.reshape(4, 128).T, f32),
        "pkwT": np.ascontiguousarray(
            pk_w.T.reshape(4, 128, 8).transpose(1, 0, 2), f32),
        "pkb": np.ascontiguousarray(pk_b.reshape(1, 8), f32),
    }
    ii = np.arange(128)
    mask = ((ii[:, None] // 8) == (ii[None, :] // 8)).astype(bf16)
    common["mask"] = mask
    # maskh[c, half, g] = 1 if c//8 == g(+16*half pattern): the two 32-wide
    # halves used to build the stage1 pair weights [W1_t | 0] and [0 | W1_t]
    maskh = np.zeros((128, 2, 32), bf16)
    m16 = ((ii[:, None] // 8) == np.arange(16)[None, :]).astype(bf16)
    maskh[:, 0, 0:16] = m16
    maskh[:, 1, 16:32] = m16
    common["maskh"] = maskh
    # W2[zrow = 32k+16b+g, m] = 1 if g == m//8  (sums the 8 tap blocks)
    zg = (ii % 32) % 16
    common["w2"] = (zg[:, None] == (ii[None, :] // 8)).astype(bf16)
    common["repl8"] = (np.arange(8)[:, None] == (ii[None, :] % 8)).astype(f32)

    in_maps = []
    for i in range(N_CORES):
        x = content_in[NSAMP * i: NSAMP * (i + 1)]
        se = style_encoding[NSAMP * i: NSAMP * (i + 1)]
        in_maps.append({
            "xraw": np.ascontiguousarray(
                x.reshape(NSAMP * 4, 128, H * W), f32),
            # style[c_lo, s, kt, yx] = se[s, kt*128+c_lo, yx]
            "style": np.ascontiguousarray(
                se.reshape(NSAMP, 4, 128, 16).transpose(2, 0, 1, 3), f32),
            **common,
        })
    return in_maps


def kernel(style_encoding, content_in, dw_w, dw_b, pk_w, pk_b, pb_w, pb_b):
    global LAST_RESULTS
    import os
    if "nc" not in _CACHE:
        _CACHE["nc"] = _build()
    nc = _CACHE["nc"]
    in_maps = _host_prep(style_encoding, content_in, dw_w, dw_b,
                         pk_w, pk_b, pb_w, pb_b)
    res = run_bass_kernel_spmd(
        nc, in_maps, core_ids=list(range(N_CORES)),
        trace=bool(os.environ.get("ADACONV_TRACE")))
    LAST_RESULTS = res
    outs = []
    for i in range(N_CORES):
        o = np.asarray(res.results[i]["out"]).astype(np.float32)
        outs.append(o.reshape(NSAMP, 4, 128, 64, 64).reshape(NSAMP, CH, 64, 64))
    return np.concatenate(outs, axis=0)
